# revision 29
# baseline (speedup 1.0000x reference)
"""Bass/Trainium2 kernel for nn_EnhancedPEFTWindowAttention.

Data-parallel over B_ (2048 windows*batch) across 8 NeuronCores:
256 windows = 12544 tokens per core. Weights / bias tables replicated.

Layout strategy (per core):
  - x pre-transposed on host to channel-rows [128, 3, T] bf16 (and a
    zero-padded fp8 copy [128, 4, T] for DoubleRow gate matmuls) so every
    linear-layer matmul contracts over the partition dim.
  - qkv LoRA-linear in channel-rows -> qkvT [128, 9, T] bf16. Gate path
    runs in fp8e4 DoubleRow (weights 64x-scaled on host, sigmoid applies
    scale=1/64). Bias folds into the PSUM->SBUF combine via
    scalar_tensor_tensor's per-partition scalar.
  - Attention packs a window PAIR (98 tokens <= 128 partitions) per score
    matmul: S^T[98 keys, 98 queries] per head; cross-window blocks are
    killed by preloading PSUM with bias(+mask) logits that are -1e30
    off-diagonal, QK matmuls accumulate (start=False), exp reads PSUM
    directly. AV contracts K=98 with a ones-column in V for row sums.
  - proj LoRA-linear in channel-rows -> outT [128, 3, T] f32; host
    un-transposes.
"""

import sys

sys.path.insert(0, "/opt/trn_rl_repo")

import numpy as np
import ml_dtypes

import concourse.bacc as bacc
import concourse.tile as tile
from concourse import mybir
from concourse.bass_utils import run_bass_kernel_spmd

BF16 = ml_dtypes.bfloat16
FP8 = ml_dtypes.float8_e4m3

WS = 7
N = 49
H = 12
D = 384
HD = 32
NW = 64
B_ = 2048
R = 16
SCALING = 32.0 / 16.0
SCALE = HD ** -0.5
GS = 64.0                      # fp8 gate-weight scale

NCORES = 8
WPC = B_ // NCORES            # windows per core = 256
TPC = WPC * N                 # tokens per core = 12544
WCHUNK = 8                    # windows per chunk
TCHUNK = WCHUNK * N           # 392 tokens per chunk
NCHUNK = WPC // WCHUNK        # 32 chunks
NPAIR = WCHUNK // 2           # 4 pairs per chunk
NP2 = 2 * N                   # tokens per pair = 98

F32 = mybir.dt.float32
BF = mybir.dt.bfloat16
F8 = mybir.dt.float8e4
DR = mybir.MatmulPerfMode.DoubleRow
ADD = mybir.AluOpType.add
MULT = mybir.AluOpType.mult
EXP = mybir.ActivationFunctionType.Exp
TANH = mybir.ActivationFunctionType.Tanh

USE_DR = True        # fp8 DoubleRow gate matmuls
# Bias(+mask) logits land in PSUM via a PE matmul (ident stationary,
# start=True) BEFORE the QK matmuls accumulate (start=False). PE-written
# PSUM sets has_written, so accumulation is HW-correct (unlike the old
# DVE-preload attempt).
QK_BANKS = 4         # 4: bank = PE row group (safe); 3: latin rounds
PAD = 32             # stationary col padding to 128 for FWL

_COMPILED = {}


def _build(has_mask: bool):
    nc = bacc.Bacc("TRN2", target_bir_lowering=False, debug=False,
                   num_devices=NCORES)

    def din(name, shape, dt):
        return nc.dram_tensor(name, shape, dt, kind="ExternalInput").ap()

    xt_d = din("xt", [128, 3, TPC], BF)
    xf8_d = din("xf8", [128, 4, TPC], F8)
    wqkvT_d = din("wqkvT", [128, 3, 3 * D], BF)
    wgf8_d = din("wgf8", [128, 4, 3 * D], F8)
    wgT_d = din("wgT", [128, 3, 3 * D], BF)
    downT_d = din("downT", [128, 3, R], BF)
    upT_d = din("upT", [R, 3 * D], BF)
    pwT_d = din("pwT", [128, 3, D], BF)
    pgT_d = din("pgT", [128, 3, D], BF)
    pdownT_d = din("pdownT", [128, 3, R], BF)
    pupT_d = din("pupT", [R, D], BF)
    bcols_d = din("bcols", [128, 12], F32)
    n_bm = 32 if has_mask else 1
    SB = QK_BANKS
    SW = (12 // SB) * NP2          # used score cols per bank
    h_bank = (lambda h: h // 4) if SB == 3 else (lambda h: h % 4)
    h_slot = (lambda h: h % 4) if SB == 3 else (lambda h: h // 4)
    # exp(bias+mask) factors, host-padded with 32 zero cols per bank
    expb_d = din("expb", [n_bm, NP2, SB, SW + PAD], BF)
    ident_d = din("ident", [128, 128], BF)
    outT_d = nc.dram_tensor("outT", [128, 3, TPC], F32,
                            kind="ExternalOutput").ap()

    with tile.TileContext(nc) as tc:
        consts = tc.alloc_tile_pool(name="consts", bufs=1)
        xt_p = tc.alloc_tile_pool(name="xt", bufs=2)
        xf8_p = tc.alloc_tile_pool(name="xf8", bufs=2)
        qkvT_p = tc.alloc_tile_pool(name="qkvT", bufs=2)
        sb_p = tc.alloc_tile_pool(name="sb", bufs=3)
        attn_p = tc.alloc_tile_pool(name="attn", bufs=2)
        oT_p = tc.alloc_tile_pool(name="oT", bufs=2)
        out_p = tc.alloc_tile_pool(name="out", bufs=2)
        ps_lin = tc.alloc_tile_pool(name="ps_lin", bufs=(2 if SB == 4 else 3),
                                    space="PSUM")
        ps_s_p = tc.alloc_tile_pool(name="ps_s", bufs=1, space="PSUM")
        ps_o_p = tc.alloc_tile_pool(name="ps_o", bufs=1, space="PSUM")
        ps_vt_p = tc.alloc_tile_pool(name="ps_vt", bufs=1, space="PSUM")

        # ---- resident constants ----
        wqkvT = consts.tile([128, 3, 3 * D], BF)
        nc.sync.dma_start(out=wqkvT, in_=wqkvT_d[:])
        if USE_DR:
            wgf8 = consts.tile([128, 4, 3 * D], F8)
            nc.sync.dma_start(out=wgf8, in_=wgf8_d[:])
        else:
            wgT = consts.tile([128, 3, 3 * D], BF)
            nc.sync.dma_start(out=wgT, in_=wgT_d[:])
        downT = consts.tile([128, 3, R], BF)
        nc.sync.dma_start(out=downT, in_=downT_d[:])
        upT = consts.tile([R, 3 * D], BF)
        nc.sync.dma_start(out=upT, in_=upT_d[:])
        pwT = consts.tile([128, 3, D], BF)
        nc.sync.dma_start(out=pwT, in_=pwT_d[:])
        pgT = consts.tile([128, 3, D], BF)
        nc.sync.dma_start(out=pgT, in_=pgT_d[:])
        pdownT = consts.tile([128, 3, R], BF)
        nc.sync.dma_start(out=pdownT, in_=pdownT_d[:])
        pupT = consts.tile([R, D], BF)
        nc.sync.dma_start(out=pupT, in_=pupT_d[:])
        bcols = consts.tile([128, 12], F32)
        nc.sync.dma_start(out=bcols, in_=bcols_d[:])
        ident = consts.tile([128, 128], BF)
        nc.sync.dma_start(out=ident, in_=ident_d[:])
        expb_c = None
        if not has_mask:
            expb_c = consts.tile([NP2, SB, SW + PAD], BF)
            nc.sync.dma_start(out=expb_c, in_=expb_d[0])

        v_exts = []
        for i in range(2):
            ve = consts.tile([NP2, H, HD + 1], BF, name=f"v_ext{i}")
            nc.vector.memset(ve[:, :, HD:HD + 1], 1.0)
            v_exts.append(ve)
        o_sbs = []
        for i in range(2):
            ob = consts.tile([NP2, H, HD], BF, name=f"o_sb{i}")
            o_sbs.append(ob)

        qkvT_tiles = {}
        oT_tiles = {}
        ep_effs = {}

        def emit_A(c):
            """qkv LoRA-linear for chunk c (channel-rows)."""
            t0 = c * TCHUNK
            xt = xt_p.tile([128, 3, TCHUNK], BF, name="xt")
            nc.sync.dma_start(out=xt, in_=xt_d[:, :, t0:t0 + TCHUNK])
            xf8 = None
            if USE_DR:
                xf8 = xf8_p.tile([128, 4, TCHUNK], F8, name="xf8")
                nc.sync.dma_start(out=xf8, in_=xf8_d[:, :, t0:t0 + TCHUNK])
            # +PAD cols so QK/VT stationaries can be 128 wide (FWL)
            qkvT = qkvT_p.tile([128, 9, TCHUNK + PAD], BF, name="qkvT")
            nc.vector.memset(qkvT[:, :, TCHUNK:TCHUNK + PAD], 0.0)
            qkvT_tiles[c] = qkvT

            # xd^T = down @ x^T  [16, TCHUNK]
            ps_xd = ps_lin.tile([128, 512], F32, tag="lin",
                                name="ps_xd")[0:R, 0:TCHUNK]
            for ki in range(3):
                nc.tensor.matmul(ps_xd, downT[:, ki, :], xt[:, ki, :],
                                 start=(ki == 0), stop=(ki == 2))
            xd = sb_p.tile([R, TCHUNK], BF, tag="xd", name="xd")
            nc.scalar.copy(xd[:], ps_xd[:])

            for mi in range(9):
                ps_g = ps_lin.tile([128, 512], F32, tag="lin",
                                   name="ps_g")[:, 0:TCHUNK]
                if USE_DR:
                    for j in range(2):
                        nc.tensor.matmul(ps_g,
                                         wgf8[:, 2 * j:2 * j + 2,
                                              128 * mi:128 * mi + 128],
                                         xf8[:, 2 * j:2 * j + 2, :],
                                         start=(j == 0), stop=(j == 1),
                                         perf_mode=DR)
                else:
                    for ki in range(3):
                        nc.tensor.matmul(ps_g,
                                         wgT[:, ki, 128 * mi:128 * mi + 128],
                                         xt[:, ki, :],
                                         start=(ki == 0), stop=(ki == 2))
                # sigmoid(z) = 0.5*(1 + tanh(z/2)); tanh shares the exp
                # activation table (no ACT_TABLE_LOAD ping-pong). The 0.5
                # is folded into up_eff on the host.
                g = sb_p.tile([128, TCHUNK], BF, tag="g", name="g")
                nc.scalar.activation(g[:], ps_g[:], TANH,
                                     scale=(0.5 / GS if USE_DR else 0.5))

                ps_l = ps_lin.tile([128, 512], F32, tag="lin",
                                   name="ps_l")[:, 0:TCHUNK]
                nc.tensor.matmul(ps_l, upT[:, 128 * mi:128 * mi + 128], xd[:],
                                 start=True, stop=True)

                ps_m = ps_lin.tile([128, 512], F32, tag="lin",
                                   name="ps_m")[:, 0:TCHUNK]
                for ki in range(3):
                    nc.tensor.matmul(ps_m,
                                     wqkvT[:, ki, 128 * mi:128 * mi + 128],
                                     xt[:, ki, :],
                                     start=(ki == 0), stop=(ki == 2))
                gl = sb_p.tile([128, TCHUNK], BF, tag="gl", name="gl")
                # gl = (tanh + 1) * (0.5*lora)
                nc.vector.scalar_tensor_tensor(gl[:], g[:], 1.0, ps_l[:],
                                               ADD, MULT)
                # qkvT = (ps_m + bias_col) + g*lora
                nc.vector.scalar_tensor_tensor(
                    qkvT[:, mi, 0:TCHUNK], ps_m[:], bcols[:, mi:mi + 1],
                    gl[:], ADD, ADD)

        def emit_B_front(c, p):
            """attention pair p of chunk c: scores, exp, expb fold, V^T."""
            qkvT = qkvT_tiles[c]
            if p == 0:
                oT_tiles[c] = oT_p.tile([128, 3, TCHUNK], BF, name="oT")
            pc0 = p * NP2

            # One PSUM tile (bank) per score group so pair p+1's QK only
            # waits on exp of pair p for that bank. Score rows 98-127 are
            # zeros (FWL-padded stationaries) and ignored.
            pss = [ps_s_p.tile([128, 512], F32, tag=f"s{b}", name=f"ps_s{b}")
                   for b in range(SB)]
            if has_mask:
                bm = attn_p.tile([NP2, SB, SW + PAD], BF, tag="bm", name="bm")
                pm = (c * NPAIR + p) % 32
                nc.sync.dma_start(out=bm, in_=expb_d[pm])
            else:
                bm = expb_c

            # QK: one matmul per head, S^T[key, query]. With SB=4 the PSUM
            # bank equals the PE row group, so the 4 heads of a round run
            # concurrently in disjoint PE row groups + PSUM banks.
            if SB == 4:
                order = list(range(12))
            else:
                order = [4 * ((r + b) % 4) + b for r in range(4)
                         for b in range(3)]
            for h in order:
                s = h % 4
                nc.tensor.matmul(
                    pss[h_bank(h)][:, 98 * h_slot(h):98 * h_slot(h) + 98],
                    qkvT[32 * s:32 * s + 32, 3 + h // 4, pc0:pc0 + 128],
                    qkvT[32 * s:32 * s + 32, h // 4, pc0:pc0 + NP2],
                    start=True, stop=True,
                    tile_position=(32 * s, 0),
                    skip_group_check=True)

            # V transpose -> key-rows [98, 12, 32] (+ ones col for rowsum)
            v_ext = v_exts[p % 2]
            ps_vt = ps_vt_p.tile([128, 3, 128], BF, tag="vt", name="ps_vt")
            for ki in range(3):
                nc.tensor.transpose(ps_vt[:, ki, :],
                                    qkvT[:, 6 + ki, pc0:pc0 + 128],
                                    ident[:])
            nc.vector.tensor_copy(
                v_ext[:, :, 0:HD],
                ps_vt[0:NP2].rearrange("p a (b c) -> p (a b) c", c=HD))

            # exp per bank, then fold exp(bias+mask) on the idle GpSimd
            # engine (cross-window blocks get expb=0). ep_eff has 32 zero
            # pad cols per bank so AV stationaries can be 128 wide (FWL).
            ep = attn_p.tile([NP2, SB, SW], BF, tag="ep", name="ep")
            ep_eff = attn_p.tile([NP2, SB, SW + PAD], BF, tag="ef",
                                 name="ep_eff")
            for b in range(SB):
                nc.scalar.activation(ep[:, b, :], pss[b][0:NP2, 0:SW], EXP)
            nc.gpsimd.memset(ep_eff[:, :, SW:SW + PAD], 0.0)
            nc.gpsimd.tensor_tensor(ep_eff[:, :, 0:SW], ep[:],
                                    bm[:, :, 0:SW], MULT)
            ep_effs[p % 2] = ep_eff

        def emit_B_av(c, p):
            """AV + normalize for pair p (emitted after front(p+1) so the
            exp -> gpsimd chain is hidden behind pair p+1's QK)."""
            ep_eff = ep_effs[p % 2]
            v_ext = v_exts[p % 2]
            # AV: one matmul per head, K=98; col 32 accumulates row sums.
            # Stationary ep_eff slice is 128 cols (FWL); output rows
            # 98-127 are garbage and ignored.
            ps_o_t = ps_o_p.tile([128, 512], F32, tag="o", name="ps_o")
            pov = ps_o_t[:, 0:H * (HD + 1)].rearrange("p (h c) -> p h c",
                                                      c=HD + 1)
            for h in range(H):
                nc.tensor.matmul(
                    pov[:, h, :],
                    ep_eff[:, h_bank(h), 98 * h_slot(h):98 * h_slot(h) + 128],
                    v_ext[:, h, :],
                    start=True, stop=True)

            r_t = attn_p.tile([NP2, H, 1], F32, tag="r", name="r")
            nc.vector.reciprocal(r_t[:], pov[0:NP2, :, HD:HD + 1])
            o_sb = o_sbs[p % 2]
            nc.vector.tensor_tensor(o_sb[:], pov[0:NP2, :, 0:HD],
                                    r_t.to_broadcast([NP2, H, HD]), MULT)

        def emit_B_back(c, p):
            """O^T transposes for pair p -> oT channel-rows."""
            oT = oT_tiles[c]
            pc0 = p * NP2
            o_sb = o_sbs[p % 2]
            ps_ot = ps_vt_p.tile([128, 3, 128], BF, tag="vt", name="ps_ot")
            for ki in range(3):
                nc.tensor.transpose(ps_ot[:, ki, 0:NP2],
                                    o_sb[:, 4 * ki:4 * ki + 4, :],
                                    ident[0:NP2, 0:NP2])
            nc.scalar.copy(oT[:, :, pc0:pc0 + NP2], ps_ot[:, :, 0:NP2])

        def emit_B(c):
            # software-pipeline: AV of pair p after front of p+1 (hides the
            # exp->gpsimd chain behind QK), OT of p after AV of p+1.
            emit_B_front(c, 0)
            emit_B_front(c, 1)
            emit_B_av(c, 0)
            emit_B_front(c, 2)
            emit_B_av(c, 1)
            emit_B_back(c, 0)
            emit_B_front(c, 3)
            emit_B_av(c, 2)
            emit_B_back(c, 1)
            emit_B_av(c, 3)
            emit_B_back(c, 2)
            emit_B_back(c, 3)

        def emit_C(c):
            """proj LoRA-linear for chunk c."""
            t0 = c * TCHUNK
            oT = oT_tiles.pop(c)
            qkvT_tiles.pop(c, None)
            ps_pxd = ps_lin.tile([128, 512], F32, tag="lin",
                                 name="ps_pxd")[0:R, 0:TCHUNK]
            for ki in range(3):
                nc.tensor.matmul(ps_pxd, pdownT[:, ki, :], oT[:, ki, :],
                                 start=(ki == 0), stop=(ki == 2))
            pxd = sb_p.tile([R, TCHUNK], BF, tag="xd", name="pxd")
            nc.scalar.copy(pxd[:], ps_pxd[:])

            out_sb = out_p.tile([128, 3, TCHUNK], F32, name="out_sb")
            for mi in range(3):
                ps_g2 = ps_lin.tile([128, 512], F32, tag="lin",
                                    name="ps_g2")[:, 0:TCHUNK]
                for ki in range(3):
                    nc.tensor.matmul(ps_g2,
                                     pgT[:, ki, 128 * mi:128 * mi + 128],
                                     oT[:, ki, :],
                                     start=(ki == 0), stop=(ki == 2))
                g2 = sb_p.tile([128, TCHUNK], BF, tag="g", name="g2")
                nc.scalar.activation(g2[:], ps_g2[:], TANH, scale=0.5)

                ps_l2 = ps_lin.tile([128, 512], F32, tag="lin",
                                    name="ps_l2")[:, 0:TCHUNK]
                nc.tensor.matmul(ps_l2, pupT[:, 128 * mi:128 * mi + 128],
                                 pxd[:], start=True, stop=True)

                ps_m2 = ps_lin.tile([128, 512], F32, tag="lin",
                                    name="ps_m2")[:, 0:TCHUNK]
                for ki in range(3):
                    nc.tensor.matmul(ps_m2,
                                     pwT[:, ki, 128 * mi:128 * mi + 128],
                                     oT[:, ki, :],
                                     start=(ki == 0), stop=(ki == 2))
                gl2 = sb_p.tile([128, TCHUNK], BF, tag="gl", name="gl2")
                nc.vector.scalar_tensor_tensor(gl2[:], g2[:], 1.0, ps_l2[:],
                                               ADD, MULT)
                nc.vector.scalar_tensor_tensor(
                    out_sb[:, mi, :], ps_m2[:], bcols[:, 9 + mi:10 + mi],
                    gl2[:], ADD, ADD)

            nc.sync.dma_start(out=outT_d[:, :, t0:t0 + TCHUNK], in_=out_sb)

        for c in range(NCHUNK):
            emit_A(c)
            if c > 0:
                emit_B(c - 1)
                emit_C(c - 1)
        emit_B(NCHUNK - 1)
        emit_C(NCHUNK - 1)

        for pool in reversed((consts, xt_p, xf8_p, qkvT_p, sb_p, attn_p,
                              oT_p, out_p, ps_lin, ps_s_p, ps_o_p, ps_vt_p)):
            pool.release()

    nc.compile()
    return nc


def _get_nc(has_mask: bool):
    key = (has_mask, USE_DR)
    if key not in _COMPILED:
        _COMPILED[key] = _build(has_mask)
    return _COMPILED[key]


def _arr_lhsT(w_t, kparts):
    """[K, M] -> [128, K//128, M] partition-tiled lhsT layout."""
    K, M = w_t.shape
    return np.ascontiguousarray(
        w_t.reshape(kparts, 128, M).transpose(1, 0, 2))


def _prep_inputs(x, mask, qkv_w, qkv_b, qkv_down, qkv_up, qkv_gate, qkv_res,
                 proj_w, proj_b, proj_down, proj_up, proj_gate, proj_res,
                 bias_table, rel_index):
    x = np.asarray(x, np.float32)
    mask = np.asarray(mask, np.float32)
    has_mask = bool(np.any(mask))

    w_eff = (np.asarray(qkv_w, np.float32)
             + np.asarray(qkv_res, np.float32))        # [1152, 384]
    # 0.5 factor: gate computed as 0.5*(1+tanh(z/2)) on device; the 0.5
    # is folded into the lora-up weights here.
    up_eff = np.asarray(qkv_up, np.float32) * (SCALING * 0.5)  # [1152, 16]
    b_eff = np.asarray(qkv_b, np.float32).copy()
    # fold attention scale into the q-channel outputs
    w_eff[0:D] *= SCALE
    up_eff[0:D] *= SCALE
    b_eff[0:D] *= SCALE

    pw_eff = (np.asarray(proj_w, np.float32)
              + np.asarray(proj_res, np.float32))
    pup_eff = np.asarray(proj_up, np.float32) * (SCALING * 0.5)

    # fp8 gate weights, 64x-scaled, K padded 384 -> 512 for DoubleRow
    gate_t = np.asarray(qkv_gate, np.float32).T * GS    # [384, 1152]
    gpad = np.zeros((512, 3 * D), np.float32)
    gpad[0:D] = gate_t
    wgf8 = np.ascontiguousarray(
        gpad.reshape(4, 128, 3 * D).transpose(1, 0, 2)).astype(FP8)

    bcols = np.zeros((128, 12), np.float32)
    bcols[:, 0:9] = b_eff.reshape(9, 128).T
    bcols[:, 9:12] = np.asarray(proj_b, np.float32).reshape(3, 128).T

    common = {
        "wqkvT": _arr_lhsT(w_eff.T, 3).astype(BF16),
        "wgf8": wgf8,
        "wgT": _arr_lhsT(np.asarray(qkv_gate, np.float32).T, 3).astype(BF16),
        "downT": _arr_lhsT(np.asarray(qkv_down, np.float32).T, 3).astype(BF16),
        "upT": np.ascontiguousarray(up_eff.T).astype(BF16),
        "pwT": _arr_lhsT(pw_eff.T, 3).astype(BF16),
        "pgT": _arr_lhsT(np.asarray(proj_gate, np.float32).T, 3).astype(BF16),
        "pdownT": _arr_lhsT(np.asarray(proj_down, np.float32).T, 3).astype(BF16),
        "pupT": np.ascontiguousarray(pup_eff.T).astype(BF16),
        "bcols": bcols,
        "ident": np.eye(128, dtype=BF16),
    }

    # score-logit preload in S^T layout: bmask[m, b, 98*s + n] for head
    # h = 4*b + s; -1e30 on cross-window blocks
    bt = np.asarray(bias_table, np.float32)
    ri = np.asarray(rel_index).astype(np.int64)
    b_nmh = bt[ri]                                # [n, m, H]

    def _bm(lg0, lg1):
        """lg_w [n, m, H] per window -> [98, SB, SW] preload tile."""
        big = np.full((NP2, NP2, H), -1e30, np.float32)   # [m, n, H]
        big[0:N, 0:N] = lg0.transpose(1, 0, 2)
        big[N:NP2, N:NP2] = lg1.transpose(1, 0, 2)
        mhn = big.transpose(0, 2, 1)                      # [m, H, n]
        if QK_BANKS == 3:
            # h = 4*bank + slot -> [m, bank, slot, n]
            arr = mhn.reshape(NP2, 3, 4, NP2)
        else:
            # h = 4*slot + bank -> [m, slot, bank, n] -> [m, bank, slot, n]
            arr = mhn.reshape(NP2, 3, 4, NP2).transpose(0, 2, 1, 3)
        sw = (12 // QK_BANKS) * NP2
        return np.ascontiguousarray(arr.reshape(NP2, QK_BANKS, sw))

    sw = (12 // QK_BANKS) * NP2
    if has_mask:
        bmask = np.zeros((32, NP2, QK_BANKS, sw), np.float32)
        for pm in range(32):
            lg0 = b_nmh + mask[2 * pm][:, :, None]
            lg1 = b_nmh + mask[2 * pm + 1][:, :, None]
            bmask[pm] = _bm(lg0, lg1)
    else:
        bmask = _bm(b_nmh, b_nmh)[None]
    expb = np.zeros(bmask.shape[:-1] + (sw + 32,), np.float32)
    expb[..., 0:sw] = np.exp(np.minimum(bmask, 80.0))
    common["expb"] = expb.astype(BF16)

    in_maps = []
    for core in range(NCORES):
        tok = np.ascontiguousarray(
            x[core * WPC:(core + 1) * WPC].reshape(TPC, D))
        xt = np.ascontiguousarray(
            tok.reshape(TPC, 3, 128).transpose(2, 1, 0)).astype(BF16)
        tpad = np.zeros((TPC, 512), np.float32)
        tpad[:, 0:D] = tok
        xf8 = np.ascontiguousarray(
            tpad.reshape(TPC, 4, 128).transpose(2, 1, 0)).astype(FP8)
        m = dict(common)
        m["xt"] = xt
        m["xf8"] = xf8
        in_maps.append(m)
    return has_mask, in_maps


def kernel(**inputs):
    has_mask, in_maps = _prep_inputs(**inputs)
    nc = _get_nc(has_mask)
    res = run_bass_kernel_spmd(nc, in_maps, list(range(NCORES)))
    outs = []
    for core in range(NCORES):
        ot = res.results[core]["outT"]            # [128, 3, TPC] f32
        out = np.ascontiguousarray(ot.transpose(2, 1, 0)).reshape(TPC, D)
        outs.append(out)
    full = np.concatenate(outs, axis=0).reshape(B_, N, D)
    return full.astype(np.float32)


def run_traced(**inputs):
    """Like kernel() but with NTFF profiling; returns (out, BassKernelResults)."""
    sys.path.insert(0, "/root/problem")
    import profhook
    profhook.install()
    has_mask, in_maps = _prep_inputs(**inputs)
    nc = _get_nc(has_mask)
    res = run_bass_kernel_spmd(nc, in_maps, list(range(NCORES)), trace=True)
    outs = []
    for core in range(NCORES):
        ot = res.results[core]["outT"]
        out = np.ascontiguousarray(ot.transpose(2, 1, 0)).reshape(TPC, D)
        outs.append(out)
    full = np.concatenate(outs, axis=0).reshape(B_, N, D)
    return full.astype(np.float32), res



# revision 34
# speedup vs baseline: 1.1982x; 1.1982x over previous
"""Bass/Trainium2 kernel for nn_EnhancedPEFTWindowAttention.

Data-parallel over B_ (2048 windows*batch) across 8 NeuronCores:
256 windows = 12544 tokens per core. Weights / bias tables replicated.

Layout strategy (per core):
  - x pre-transposed on host to channel-rows [128, 3, T] bf16 (and a
    zero-padded fp8 copy [128, 4, T] for DoubleRow gate matmuls) so every
    linear-layer matmul contracts over the partition dim.
  - qkv LoRA-linear in channel-rows -> qkvT [128, 9, T] bf16. Gate path
    runs in fp8e4 DoubleRow (weights 64x-scaled on host, sigmoid applies
    scale=1/64). Bias folds into the PSUM->SBUF combine via
    scalar_tensor_tensor's per-partition scalar.
  - Attention packs a window PAIR (98 tokens <= 128 partitions) per score
    matmul: S^T[98 keys, 98 queries] per head; cross-window blocks are
    killed by preloading PSUM with bias(+mask) logits that are -1e30
    off-diagonal, QK matmuls accumulate (start=False), exp reads PSUM
    directly. AV contracts K=98 with a ones-column in V for row sums.
  - proj LoRA-linear in channel-rows -> outT [128, 3, T] f32; host
    un-transposes.
"""

import sys

sys.path.insert(0, "/opt/trn_rl_repo")

import numpy as np
import ml_dtypes

import concourse.bacc as bacc
import concourse.tile as tile
from concourse import mybir
from concourse.bass_utils import run_bass_kernel_spmd

BF16 = ml_dtypes.bfloat16
FP8 = ml_dtypes.float8_e4m3

WS = 7
N = 49
H = 12
D = 384
HD = 32
NW = 64
B_ = 2048
R = 16
SCALING = 32.0 / 16.0
SCALE = HD ** -0.5
GS = 64.0                      # fp8 gate-weight scale

NCORES = 8
WPC = B_ // NCORES            # windows per core = 256
TPC = WPC * N                 # tokens per core = 12544
WCHUNK = 8                    # windows per chunk
TCHUNK = WCHUNK * N           # 392 tokens per chunk
NCHUNK = WPC // WCHUNK        # 32 chunks
NPAIR = WCHUNK // 2           # 4 pairs per chunk
NP2 = 2 * N                   # tokens per pair = 98

F32 = mybir.dt.float32
BF = mybir.dt.bfloat16
F8 = mybir.dt.float8e4
DR = mybir.MatmulPerfMode.DoubleRow
ADD = mybir.AluOpType.add
MULT = mybir.AluOpType.mult
EXP = mybir.ActivationFunctionType.Exp
TANH = mybir.ActivationFunctionType.Tanh

USE_DR = True        # fp8 DoubleRow gate matmuls
# Bias(+mask) logits land in PSUM via a PE matmul (ident stationary,
# start=True) BEFORE the QK matmuls accumulate (start=False). PE-written
# PSUM sets has_written, so accumulation is HW-correct (unlike the old
# DVE-preload attempt).
QK_BANKS = 4         # 4: bank = PE row group (safe); 3: latin rounds
PAD = 32             # stationary col padding to 128 for FWL

_COMPILED = {}


def _build(has_mask: bool):
    nc = bacc.Bacc("TRN2", target_bir_lowering=False, debug=False,
                   num_devices=NCORES)

    def din(name, shape, dt):
        return nc.dram_tensor(name, shape, dt, kind="ExternalInput").ap()

    xt_d = din("xt", [128, 3, TPC], BF)
    xf8_d = din("xf8", [128, 4, TPC], F8)
    wqkvT_d = din("wqkvT", [128, 3, 3 * D], BF)
    wgf8_d = din("wgf8", [128, 4, 3 * D], F8)
    wgT_d = din("wgT", [128, 3, 3 * D], BF)
    downT_d = din("downT", [128, 3, R], BF)
    upT_d = din("upT", [R, 3 * D], BF)
    pwT_d = din("pwT", [128, 3, D], BF)
    pgT_d = din("pgT", [128, 3, D], BF)
    pdownT_d = din("pdownT", [128, 3, R], BF)
    pupT_d = din("pupT", [R, D], BF)
    bcols_d = din("bcols", [128, 12], F32)
    n_bm = 32 if has_mask else 1
    SB = QK_BANKS
    SW = (12 // SB) * NP2          # used score cols per bank
    h_bank = (lambda h: h // 4) if SB == 3 else (lambda h: h % 4)
    h_slot = (lambda h: h % 4) if SB == 3 else (lambda h: h // 4)
    # bias+mask logits (bf16), fed to PSUM via a PE ident-matmul preload
    bmask_d = din("bmask", [n_bm, NP2, SB, SW], BF)
    ident_d = din("ident", [128, 128], BF)
    outT_d = nc.dram_tensor("outT", [128, 3, TPC], F32,
                            kind="ExternalOutput").ap()

    with tile.TileContext(nc) as tc:
        consts = tc.alloc_tile_pool(name="consts", bufs=1)
        xt_p = tc.alloc_tile_pool(name="xt", bufs=2)
        xf8_p = tc.alloc_tile_pool(name="xf8", bufs=2)
        qkvT_p = tc.alloc_tile_pool(name="qkvT", bufs=2)
        sb_p = tc.alloc_tile_pool(name="sb", bufs=3)
        attn_p = tc.alloc_tile_pool(name="attn", bufs=2)
        oT_p = tc.alloc_tile_pool(name="oT", bufs=2)
        out_p = tc.alloc_tile_pool(name="out", bufs=2)
        ps_lin = tc.alloc_tile_pool(name="ps_lin", bufs=(2 if SB == 4 else 3),
                                    space="PSUM")
        ps_s_p = tc.alloc_tile_pool(name="ps_s", bufs=1, space="PSUM")
        ps_o_p = tc.alloc_tile_pool(name="ps_o", bufs=1, space="PSUM")
        ps_vt_p = tc.alloc_tile_pool(name="ps_vt", bufs=1, space="PSUM")

        # ---- resident constants ----
        wqkvT = consts.tile([128, 3, 3 * D], BF)
        nc.sync.dma_start(out=wqkvT, in_=wqkvT_d[:])
        if USE_DR:
            wgf8 = consts.tile([128, 4, 3 * D], F8)
            nc.sync.dma_start(out=wgf8, in_=wgf8_d[:])
        else:
            wgT = consts.tile([128, 3, 3 * D], BF)
            nc.sync.dma_start(out=wgT, in_=wgT_d[:])
        downT = consts.tile([128, 3, R], BF)
        nc.sync.dma_start(out=downT, in_=downT_d[:])
        upT = consts.tile([R, 3 * D], BF)
        nc.sync.dma_start(out=upT, in_=upT_d[:])
        pwT = consts.tile([128, 3, D], BF)
        nc.sync.dma_start(out=pwT, in_=pwT_d[:])
        pgT = consts.tile([128, 3, D], BF)
        nc.sync.dma_start(out=pgT, in_=pgT_d[:])
        pdownT = consts.tile([128, 3, R], BF)
        nc.sync.dma_start(out=pdownT, in_=pdownT_d[:])
        pupT = consts.tile([R, D], BF)
        nc.sync.dma_start(out=pupT, in_=pupT_d[:])
        bcols = consts.tile([128, 12], F32)
        nc.sync.dma_start(out=bcols, in_=bcols_d[:])
        ident = consts.tile([128, 128], BF)
        nc.sync.dma_start(out=ident, in_=ident_d[:])
        bmask_c = None
        if not has_mask:
            bmask_c = consts.tile([NP2, SB, SW], BF)
            nc.sync.dma_start(out=bmask_c, in_=bmask_d[0])

        v_exts = []
        for i in range(2):
            ve = consts.tile([NP2, H, HD + 1], BF, name=f"v_ext{i}")
            nc.vector.memset(ve[:, :, HD:HD + 1], 1.0)
            v_exts.append(ve)
        o_sbs = []
        for i in range(2):
            ob = consts.tile([NP2, H, HD], BF, name=f"o_sb{i}")
            o_sbs.append(ob)

        qkvT_tiles = {}
        oT_tiles = {}
        ep_effs = {}

        def emit_A(c):
            """qkv LoRA-linear for chunk c (channel-rows)."""
            t0 = c * TCHUNK
            xt = xt_p.tile([128, 3, TCHUNK], BF, name="xt")
            nc.sync.dma_start(out=xt, in_=xt_d[:, :, t0:t0 + TCHUNK])
            xf8 = None
            if USE_DR:
                xf8 = xf8_p.tile([128, 4, TCHUNK], F8, name="xf8")
                nc.sync.dma_start(out=xf8, in_=xf8_d[:, :, t0:t0 + TCHUNK])
            # +PAD cols so QK/VT stationaries can be 128 wide (FWL)
            qkvT = qkvT_p.tile([128, 9, TCHUNK + PAD], BF, name="qkvT")
            nc.vector.memset(qkvT[:, :, TCHUNK:TCHUNK + PAD], 0.0)
            qkvT_tiles[c] = qkvT

            # xd^T = down @ x^T  [16, TCHUNK]
            ps_xd = ps_lin.tile([128, 512], F32, tag="lin",
                                name="ps_xd")[0:R, 0:TCHUNK]
            for ki in range(3):
                nc.tensor.matmul(ps_xd, downT[:, ki, :], xt[:, ki, :],
                                 start=(ki == 0), stop=(ki == 2))
            xd = sb_p.tile([R, TCHUNK], BF, tag="xd", name="xd")
            nc.scalar.copy(xd[:], ps_xd[:])

            for mi in range(9):
                ps_g = ps_lin.tile([128, 512], F32, tag="lin",
                                   name="ps_g")[:, 0:TCHUNK]
                if USE_DR:
                    for j in range(2):
                        nc.tensor.matmul(ps_g,
                                         wgf8[:, 2 * j:2 * j + 2,
                                              128 * mi:128 * mi + 128],
                                         xf8[:, 2 * j:2 * j + 2, :],
                                         start=(j == 0), stop=(j == 1),
                                         perf_mode=DR)
                else:
                    for ki in range(3):
                        nc.tensor.matmul(ps_g,
                                         wgT[:, ki, 128 * mi:128 * mi + 128],
                                         xt[:, ki, :],
                                         start=(ki == 0), stop=(ki == 2))
                # sigmoid(z) = 0.5*(1 + tanh(z/2)); tanh shares the exp
                # activation table (no ACT_TABLE_LOAD ping-pong). The 0.5
                # is folded into up_eff on the host.
                g = sb_p.tile([128, TCHUNK], BF, tag="g", name="g")
                nc.scalar.activation(g[:], ps_g[:], TANH,
                                     scale=(0.5 / GS if USE_DR else 0.5))

                ps_l = ps_lin.tile([128, 512], F32, tag="lin",
                                   name="ps_l")[:, 0:TCHUNK]
                nc.tensor.matmul(ps_l, upT[:, 128 * mi:128 * mi + 128], xd[:],
                                 start=True, stop=True)

                ps_m = ps_lin.tile([128, 512], F32, tag="lin",
                                   name="ps_m")[:, 0:TCHUNK]
                for ki in range(3):
                    nc.tensor.matmul(ps_m,
                                     wqkvT[:, ki, 128 * mi:128 * mi + 128],
                                     xt[:, ki, :],
                                     start=(ki == 0), stop=(ki == 2))
                gl = sb_p.tile([128, TCHUNK], BF, tag="gl", name="gl")
                # gl = (tanh + 1) * (0.5*lora)
                nc.vector.scalar_tensor_tensor(gl[:], g[:], 1.0, ps_l[:],
                                               ADD, MULT)
                # qkvT = (ps_m + bias_col) + g*lora
                nc.vector.scalar_tensor_tensor(
                    qkvT[:, mi, 0:TCHUNK], ps_m[:], bcols[:, mi:mi + 1],
                    gl[:], ADD, ADD)

        def emit_B_front(c, p):
            """attention pair p of chunk c: scores, exp, expb fold, V^T."""
            qkvT = qkvT_tiles[c]
            if p == 0:
                oT_tiles[c] = oT_p.tile([128, 3, TCHUNK], BF, name="oT")
            pc0 = p * NP2

            # One PSUM tile (bank) per score group so pair p+1's QK only
            # waits on exp of pair p for that bank. Score rows 98-127 are
            # zeros (FWL-padded stationaries) and ignored.
            pss = [ps_s_p.tile([128, 512], F32, tag=f"s{b}", name=f"ps_s{b}")
                   for b in range(SB)]
            if has_mask:
                bm = attn_p.tile([NP2, SB, SW], BF, tag="bm", name="bm")
                pm = (c * NPAIR + p) % 32
                nc.sync.dma_start(out=bm, in_=bmask_d[pm])
            else:
                bm = bmask_c
            # bias(+mask) logits -> PSUM via PE (ident stationary,
            # start=True sets has_written; cross-window blocks are -1e30
            # so exp kills them after QK accumulates on top).
            for b in range(SB):
                nc.tensor.matmul(pss[b][0:NP2, 0:SW], ident[0:NP2, 0:NP2],
                                 bm[:, b, :], start=True, stop=False,
                                 skip_group_check=True)

            # QK: one matmul per head, S^T[key, query], accumulating onto
            # the bias preload. With SB=4 the PSUM bank equals the PE row
            # group, so the 4 heads of a round run concurrently in
            # disjoint PE row groups + PSUM banks.
            if SB == 4:
                order = list(range(12))
            else:
                order = [4 * ((r + b) % 4) + b for r in range(4)
                         for b in range(3)]
            for h in order:
                s = h % 4
                nc.tensor.matmul(
                    pss[h_bank(h)][:, 98 * h_slot(h):98 * h_slot(h) + 98],
                    qkvT[32 * s:32 * s + 32, 3 + h // 4, pc0:pc0 + 128],
                    qkvT[32 * s:32 * s + 32, h // 4, pc0:pc0 + NP2],
                    start=False, stop=(h_slot(h) == 2),
                    tile_position=(32 * s, 0),
                    skip_group_check=True)

            # V transpose -> key-rows [98, 12, 32] (+ ones col for rowsum)
            v_ext = v_exts[p % 2]
            ps_vt = ps_vt_p.tile([128, 3, 128], BF, tag="vt", name="ps_vt")
            for ki in range(3):
                nc.tensor.transpose(ps_vt[:, ki, :],
                                    qkvT[:, 6 + ki, pc0:pc0 + 128],
                                    ident[:])
            nc.vector.tensor_copy(
                v_ext[:, :, 0:HD],
                ps_vt[0:NP2].rearrange("p a (b c) -> p (a b) c", c=HD))

            # exp per bank, reading PSUM directly (bias already folded).
            # 32 zero pad cols per bank so AV stationaries are 128 (FWL).
            ep = attn_p.tile([NP2, SB, SW + PAD], BF, tag="ep", name="ep")
            nc.vector.memset(ep[:, :, SW:SW + PAD], 0.0)
            for b in range(SB):
                nc.scalar.activation(ep[:, b, 0:SW], pss[b][0:NP2, 0:SW],
                                     EXP)
            ep_effs[p % 2] = ep

        def emit_B_av(c, p):
            """AV + normalize for pair p (emitted after front(p+1) so the
            exp -> gpsimd chain is hidden behind pair p+1's QK)."""
            ep_eff = ep_effs[p % 2]
            v_ext = v_exts[p % 2]
            # AV: one matmul per head, K=98; col 32 accumulates row sums.
            # Stationary ep_eff slice is 128 cols (FWL); output rows
            # 98-127 are garbage and ignored.
            ps_o_t = ps_o_p.tile([128, 512], F32, tag="o", name="ps_o")
            pov = ps_o_t[:, 0:H * (HD + 1)].rearrange("p (h c) -> p h c",
                                                      c=HD + 1)
            for h in range(H):
                nc.tensor.matmul(
                    pov[:, h, :],
                    ep_eff[:, h_bank(h), 98 * h_slot(h):98 * h_slot(h) + 128],
                    v_ext[:, h, :],
                    start=True, stop=True)

            r_t = attn_p.tile([NP2, H, 1], F32, tag="r", name="r")
            nc.vector.reciprocal(r_t[:], pov[0:NP2, :, HD:HD + 1])
            o_sb = o_sbs[p % 2]
            nc.vector.tensor_tensor(o_sb[:], pov[0:NP2, :, 0:HD],
                                    r_t.to_broadcast([NP2, H, HD]), MULT)

        def emit_B_back(c, p):
            """O^T transposes for pair p -> oT channel-rows."""
            oT = oT_tiles[c]
            pc0 = p * NP2
            o_sb = o_sbs[p % 2]
            ps_ot = ps_vt_p.tile([128, 3, 128], BF, tag="vt", name="ps_ot")
            for ki in range(3):
                nc.tensor.transpose(ps_ot[:, ki, 0:NP2],
                                    o_sb[:, 4 * ki:4 * ki + 4, :],
                                    ident[0:NP2, 0:NP2])
            nc.scalar.copy(oT[:, :, pc0:pc0 + NP2], ps_ot[:, :, 0:NP2])

        def emit_B(c):
            # software-pipeline: AV of pair p after front of p+1 (hides the
            # exp->gpsimd chain behind QK), OT of p after AV of p+1.
            emit_B_front(c, 0)
            emit_B_front(c, 1)
            emit_B_av(c, 0)
            emit_B_front(c, 2)
            emit_B_av(c, 1)
            emit_B_back(c, 0)
            emit_B_front(c, 3)
            emit_B_av(c, 2)
            emit_B_back(c, 1)
            emit_B_av(c, 3)
            emit_B_back(c, 2)
            emit_B_back(c, 3)

        def emit_C(c):
            """proj LoRA-linear for chunk c."""
            t0 = c * TCHUNK
            oT = oT_tiles.pop(c)
            qkvT_tiles.pop(c, None)
            ps_pxd = ps_lin.tile([128, 512], F32, tag="lin",
                                 name="ps_pxd")[0:R, 0:TCHUNK]
            for ki in range(3):
                nc.tensor.matmul(ps_pxd, pdownT[:, ki, :], oT[:, ki, :],
                                 start=(ki == 0), stop=(ki == 2))
            pxd = sb_p.tile([R, TCHUNK], BF, tag="xd", name="pxd")
            nc.scalar.copy(pxd[:], ps_pxd[:])

            out_sb = out_p.tile([128, 3, TCHUNK], F32, name="out_sb")
            for mi in range(3):
                ps_g2 = ps_lin.tile([128, 512], F32, tag="lin",
                                    name="ps_g2")[:, 0:TCHUNK]
                for ki in range(3):
                    nc.tensor.matmul(ps_g2,
                                     pgT[:, ki, 128 * mi:128 * mi + 128],
                                     oT[:, ki, :],
                                     start=(ki == 0), stop=(ki == 2))
                g2 = sb_p.tile([128, TCHUNK], BF, tag="g", name="g2")
                nc.scalar.activation(g2[:], ps_g2[:], TANH, scale=0.5)

                ps_l2 = ps_lin.tile([128, 512], F32, tag="lin",
                                    name="ps_l2")[:, 0:TCHUNK]
                nc.tensor.matmul(ps_l2, pupT[:, 128 * mi:128 * mi + 128],
                                 pxd[:], start=True, stop=True)

                ps_m2 = ps_lin.tile([128, 512], F32, tag="lin",
                                    name="ps_m2")[:, 0:TCHUNK]
                for ki in range(3):
                    nc.tensor.matmul(ps_m2,
                                     pwT[:, ki, 128 * mi:128 * mi + 128],
                                     oT[:, ki, :],
                                     start=(ki == 0), stop=(ki == 2))
                gl2 = sb_p.tile([128, TCHUNK], BF, tag="gl", name="gl2")
                nc.vector.scalar_tensor_tensor(gl2[:], g2[:], 1.0, ps_l2[:],
                                               ADD, MULT)
                nc.vector.scalar_tensor_tensor(
                    out_sb[:, mi, :], ps_m2[:], bcols[:, 9 + mi:10 + mi],
                    gl2[:], ADD, ADD)

            nc.sync.dma_start(out=outT_d[:, :, t0:t0 + TCHUNK], in_=out_sb)

        for c in range(NCHUNK):
            emit_A(c)
            if c > 0:
                emit_B(c - 1)
                emit_C(c - 1)
        emit_B(NCHUNK - 1)
        emit_C(NCHUNK - 1)

        for pool in reversed((consts, xt_p, xf8_p, qkvT_p, sb_p, attn_p,
                              oT_p, out_p, ps_lin, ps_s_p, ps_o_p, ps_vt_p)):
            pool.release()

    nc.compile()
    return nc


def _get_nc(has_mask: bool):
    key = (has_mask, USE_DR)
    if key not in _COMPILED:
        _COMPILED[key] = _build(has_mask)
    return _COMPILED[key]


def _arr_lhsT(w_t, kparts):
    """[K, M] -> [128, K//128, M] partition-tiled lhsT layout."""
    K, M = w_t.shape
    return np.ascontiguousarray(
        w_t.reshape(kparts, 128, M).transpose(1, 0, 2))


def _prep_inputs(x, mask, qkv_w, qkv_b, qkv_down, qkv_up, qkv_gate, qkv_res,
                 proj_w, proj_b, proj_down, proj_up, proj_gate, proj_res,
                 bias_table, rel_index):
    x = np.asarray(x, np.float32)
    mask = np.asarray(mask, np.float32)
    has_mask = bool(np.any(mask))

    w_eff = (np.asarray(qkv_w, np.float32)
             + np.asarray(qkv_res, np.float32))        # [1152, 384]
    # 0.5 factor: gate computed as 0.5*(1+tanh(z/2)) on device; the 0.5
    # is folded into the lora-up weights here.
    up_eff = np.asarray(qkv_up, np.float32) * (SCALING * 0.5)  # [1152, 16]
    b_eff = np.asarray(qkv_b, np.float32).copy()
    # fold attention scale into the q-channel outputs
    w_eff[0:D] *= SCALE
    up_eff[0:D] *= SCALE
    b_eff[0:D] *= SCALE

    pw_eff = (np.asarray(proj_w, np.float32)
              + np.asarray(proj_res, np.float32))
    pup_eff = np.asarray(proj_up, np.float32) * (SCALING * 0.5)

    # fp8 gate weights, 64x-scaled, K padded 384 -> 512 for DoubleRow
    gate_t = np.asarray(qkv_gate, np.float32).T * GS    # [384, 1152]
    gpad = np.zeros((512, 3 * D), np.float32)
    gpad[0:D] = gate_t
    wgf8 = np.ascontiguousarray(
        gpad.reshape(4, 128, 3 * D).transpose(1, 0, 2)).astype(FP8)

    bcols = np.zeros((128, 12), np.float32)
    bcols[:, 0:9] = b_eff.reshape(9, 128).T
    bcols[:, 9:12] = np.asarray(proj_b, np.float32).reshape(3, 128).T

    common = {
        "wqkvT": _arr_lhsT(w_eff.T, 3).astype(BF16),
        "wgf8": wgf8,
        "wgT": _arr_lhsT(np.asarray(qkv_gate, np.float32).T, 3).astype(BF16),
        "downT": _arr_lhsT(np.asarray(qkv_down, np.float32).T, 3).astype(BF16),
        "upT": np.ascontiguousarray(up_eff.T).astype(BF16),
        "pwT": _arr_lhsT(pw_eff.T, 3).astype(BF16),
        "pgT": _arr_lhsT(np.asarray(proj_gate, np.float32).T, 3).astype(BF16),
        "pdownT": _arr_lhsT(np.asarray(proj_down, np.float32).T, 3).astype(BF16),
        "pupT": np.ascontiguousarray(pup_eff.T).astype(BF16),
        "bcols": bcols,
        "ident": np.eye(128, dtype=BF16),
    }

    # score-logit preload in S^T layout: bmask[m, b, 98*s + n] for head
    # h = 4*b + s; -1e30 on cross-window blocks
    bt = np.asarray(bias_table, np.float32)
    ri = np.asarray(rel_index).astype(np.int64)
    b_nmh = bt[ri]                                # [n, m, H]

    def _bm(lg0, lg1):
        """lg_w [n, m, H] per window -> [98, SB, SW] preload tile."""
        big = np.full((NP2, NP2, H), -1e30, np.float32)   # [m, n, H]
        big[0:N, 0:N] = lg0.transpose(1, 0, 2)
        big[N:NP2, N:NP2] = lg1.transpose(1, 0, 2)
        mhn = big.transpose(0, 2, 1)                      # [m, H, n]
        if QK_BANKS == 3:
            # h = 4*bank + slot -> [m, bank, slot, n]
            arr = mhn.reshape(NP2, 3, 4, NP2)
        else:
            # h = 4*slot + bank -> [m, slot, bank, n] -> [m, bank, slot, n]
            arr = mhn.reshape(NP2, 3, 4, NP2).transpose(0, 2, 1, 3)
        sw = (12 // QK_BANKS) * NP2
        return np.ascontiguousarray(arr.reshape(NP2, QK_BANKS, sw))

    sw = (12 // QK_BANKS) * NP2
    if has_mask:
        bmask = np.zeros((32, NP2, QK_BANKS, sw), np.float32)
        for pm in range(32):
            lg0 = b_nmh + mask[2 * pm][:, :, None]
            lg1 = b_nmh + mask[2 * pm + 1][:, :, None]
            bmask[pm] = _bm(lg0, lg1)
    else:
        bmask = _bm(b_nmh, b_nmh)[None]
    common["bmask"] = bmask.astype(BF16)

    in_maps = []
    for core in range(NCORES):
        tok = np.ascontiguousarray(
            x[core * WPC:(core + 1) * WPC].reshape(TPC, D))
        xt = np.ascontiguousarray(
            tok.reshape(TPC, 3, 128).transpose(2, 1, 0)).astype(BF16)
        tpad = np.zeros((TPC, 512), np.float32)
        tpad[:, 0:D] = tok
        xf8 = np.ascontiguousarray(
            tpad.reshape(TPC, 4, 128).transpose(2, 1, 0)).astype(FP8)
        m = dict(common)
        m["xt"] = xt
        m["xf8"] = xf8
        in_maps.append(m)
    return has_mask, in_maps


def kernel(**inputs):
    has_mask, in_maps = _prep_inputs(**inputs)
    nc = _get_nc(has_mask)
    res = run_bass_kernel_spmd(nc, in_maps, list(range(NCORES)))
    outs = []
    for core in range(NCORES):
        ot = res.results[core]["outT"]            # [128, 3, TPC] f32
        out = np.ascontiguousarray(ot.transpose(2, 1, 0)).reshape(TPC, D)
        outs.append(out)
    full = np.concatenate(outs, axis=0).reshape(B_, N, D)
    return full.astype(np.float32)


def run_traced(**inputs):
    """Like kernel() but with NTFF profiling; returns (out, BassKernelResults)."""
    sys.path.insert(0, "/root/problem")
    import profhook
    profhook.install()
    has_mask, in_maps = _prep_inputs(**inputs)
    nc = _get_nc(has_mask)
    res = run_bass_kernel_spmd(nc, in_maps, list(range(NCORES)), trace=True)
    outs = []
    for core in range(NCORES):
        ot = res.results[core]["outT"]
        out = np.ascontiguousarray(ot.transpose(2, 1, 0)).reshape(TPC, D)
        outs.append(out)
    full = np.concatenate(outs, axis=0).reshape(B_, N, D)
    return full.astype(np.float32), res



# revision 37
# speedup vs baseline: 1.2856x; 1.0729x over previous
"""Bass/Trainium2 kernel for nn_EnhancedPEFTWindowAttention.

Data-parallel over B_ (2048 windows*batch) across 8 NeuronCores:
256 windows = 12544 tokens per core. Weights / bias tables replicated.

Layout strategy (per core):
  - x pre-transposed on host to channel-rows [128, 3, T] bf16 (and a
    zero-padded fp8 copy [128, 4, T] for DoubleRow gate matmuls) so every
    linear-layer matmul contracts over the partition dim.
  - qkv LoRA-linear in channel-rows -> qkvT [128, 9, T] bf16. Gate path
    runs in fp8e4 DoubleRow (weights 64x-scaled on host, sigmoid applies
    scale=1/64). Bias folds into the PSUM->SBUF combine via
    scalar_tensor_tensor's per-partition scalar.
  - Attention packs a window PAIR (98 tokens <= 128 partitions) per score
    matmul: S^T[98 keys, 98 queries] per head; cross-window blocks are
    killed by preloading PSUM with bias(+mask) logits that are -1e30
    off-diagonal, QK matmuls accumulate (start=False), exp reads PSUM
    directly. AV contracts K=98 with a ones-column in V for row sums.
  - proj LoRA-linear in channel-rows -> outT [128, 3, T] f32; host
    un-transposes.
"""

import sys

sys.path.insert(0, "/opt/trn_rl_repo")

import numpy as np
import ml_dtypes

import concourse.bacc as bacc
import concourse.tile as tile
from concourse import mybir
from concourse.bass_utils import run_bass_kernel_spmd

BF16 = ml_dtypes.bfloat16
FP8 = ml_dtypes.float8_e4m3

WS = 7
N = 49
H = 12
D = 384
HD = 32
NW = 64
B_ = 2048
R = 16
SCALING = 32.0 / 16.0
SCALE = HD ** -0.5
GS = 64.0                      # fp8 gate-weight scale

NCORES = 8
WPC = B_ // NCORES            # windows per core = 256
TPC = WPC * N                 # tokens per core = 12544
WCHUNK = 8                    # windows per chunk
TCHUNK = WCHUNK * N           # 392 tokens per chunk
NCHUNK = WPC // WCHUNK        # 32 chunks
NPAIR = WCHUNK // 2           # 4 pairs per chunk
NP2 = 2 * N                   # tokens per pair = 98

F32 = mybir.dt.float32
BF = mybir.dt.bfloat16
F8 = mybir.dt.float8e4
DR = mybir.MatmulPerfMode.DoubleRow
ADD = mybir.AluOpType.add
MULT = mybir.AluOpType.mult
EXP = mybir.ActivationFunctionType.Exp
TANH = mybir.ActivationFunctionType.Tanh

USE_DR = True        # fp8 DoubleRow gate matmuls
# Bias(+mask) logits land in PSUM via a PE matmul (ident stationary,
# start=True) BEFORE the QK matmuls accumulate (start=False). PE-written
# PSUM sets has_written, so accumulation is HW-correct (unlike the old
# DVE-preload attempt).
QK_BANKS = 4         # 4: bank = PE row group (safe); 3: latin rounds
PAD = 32             # stationary col padding to 128 for FWL

_COMPILED = {}


def _build(has_mask: bool):
    nc = bacc.Bacc("TRN2", target_bir_lowering=False, debug=False,
                   num_devices=NCORES)

    def din(name, shape, dt):
        return nc.dram_tensor(name, shape, dt, kind="ExternalInput").ap()

    xt_d = din("xt", [128, 3, TPC], BF)
    xf8_d = din("xf8", [128, 4, TPC], F8)
    wqkvT_d = din("wqkvT", [128, 3, 3 * D], BF)
    wgf8_d = din("wgf8", [128, 4, 3 * D], F8)
    wgT_d = din("wgT", [128, 3, 3 * D], BF)
    downT_d = din("downT", [128, 3, R], BF)
    upT_d = din("upT", [R, 3 * D], BF)
    pwT_d = din("pwT", [128, 3, D], BF)
    pgT_d = din("pgT", [128, 3, D], BF)
    pdownT_d = din("pdownT", [128, 3, R], BF)
    pupT_d = din("pupT", [R, D], BF)
    bcols_d = din("bcols", [128, 12], F32)
    n_bm = 32 if has_mask else 1
    SB = QK_BANKS
    SW = (12 // SB) * NP2          # used score cols per bank
    h_bank = (lambda h: h // 4) if SB == 3 else (lambda h: h % 4)
    h_slot = (lambda h: h % 4) if SB == 3 else (lambda h: h // 4)
    # bias+mask logits (bf16), fed to PSUM via a PE ident-matmul preload
    bmask_d = din("bmask", [n_bm, NP2, SB, SW], BF)
    ident_d = din("ident", [128, 128], BF)
    outT_d = nc.dram_tensor("outT", [128, 3, TPC], F32,
                            kind="ExternalOutput").ap()

    with tile.TileContext(nc) as tc:
        consts = tc.alloc_tile_pool(name="consts", bufs=1)
        xt_p = tc.alloc_tile_pool(name="xt", bufs=2)
        xf8_p = tc.alloc_tile_pool(name="xf8", bufs=2)
        qkvT_p = tc.alloc_tile_pool(name="qkvT", bufs=2)
        sb_p = tc.alloc_tile_pool(name="sb", bufs=3)
        attn_p = tc.alloc_tile_pool(name="attn", bufs=2)
        oT_p = tc.alloc_tile_pool(name="oT", bufs=2)
        out_p = tc.alloc_tile_pool(name="out", bufs=2)
        ps_lin = tc.alloc_tile_pool(name="ps_lin", bufs=(2 if SB == 4 else 3),
                                    space="PSUM")
        ps_s_p = tc.alloc_tile_pool(name="ps_s", bufs=1, space="PSUM")
        ps_o_p = tc.alloc_tile_pool(name="ps_o", bufs=1, space="PSUM")
        ps_vt_p = tc.alloc_tile_pool(name="ps_vt", bufs=1, space="PSUM")

        # ---- resident constants ----
        wqkvT = consts.tile([128, 3, 3 * D], BF)
        nc.sync.dma_start(out=wqkvT, in_=wqkvT_d[:])
        if USE_DR:
            wgf8 = consts.tile([128, 4, 3 * D], F8)
            nc.sync.dma_start(out=wgf8, in_=wgf8_d[:])
        else:
            wgT = consts.tile([128, 3, 3 * D], BF)
            nc.sync.dma_start(out=wgT, in_=wgT_d[:])
        downT = consts.tile([128, 3, R], BF)
        nc.sync.dma_start(out=downT, in_=downT_d[:])
        upT = consts.tile([R, 3 * D], BF)
        nc.sync.dma_start(out=upT, in_=upT_d[:])
        pwT = consts.tile([128, 3, D], BF)
        nc.sync.dma_start(out=pwT, in_=pwT_d[:])
        pgT = consts.tile([128, 3, D], BF)
        nc.sync.dma_start(out=pgT, in_=pgT_d[:])
        pdownT = consts.tile([128, 3, R], BF)
        nc.sync.dma_start(out=pdownT, in_=pdownT_d[:])
        pupT = consts.tile([R, D], BF)
        nc.sync.dma_start(out=pupT, in_=pupT_d[:])
        bcols = consts.tile([128, 12], F32)
        nc.sync.dma_start(out=bcols, in_=bcols_d[:])
        ident = consts.tile([128, 128], BF)
        nc.sync.dma_start(out=ident, in_=ident_d[:])
        bmask_c = None
        if not has_mask:
            bmask_c = consts.tile([NP2, SB, SW], BF)
            nc.sync.dma_start(out=bmask_c, in_=bmask_d[0])

        v_exts = []
        for i in range(2):
            ve = consts.tile([NP2, H, HD + 1], BF, name=f"v_ext{i}")
            nc.vector.memset(ve[:, :, HD:HD + 1], 1.0)
            v_exts.append(ve)
        o_sbs = []
        for i in range(2):
            ob = consts.tile([NP2, H, HD], BF, name=f"o_sb{i}")
            o_sbs.append(ob)

        qkvT_tiles = {}
        oT_tiles = {}
        ep_effs = {}

        def emit_A(c):
            """qkv LoRA-linear for chunk c (channel-rows)."""
            t0 = c * TCHUNK
            xt = xt_p.tile([128, 3, TCHUNK], BF, name="xt")
            nc.sync.dma_start(out=xt, in_=xt_d[:, :, t0:t0 + TCHUNK])
            xf8 = None
            if USE_DR:
                xf8 = xf8_p.tile([128, 4, TCHUNK], F8, name="xf8")
                nc.sync.dma_start(out=xf8, in_=xf8_d[:, :, t0:t0 + TCHUNK])
            # +PAD cols so QK/VT stationaries can be 128 wide (FWL)
            qkvT = qkvT_p.tile([128, 9, TCHUNK + PAD], BF, name="qkvT")
            nc.vector.memset(qkvT[:, :, TCHUNK:TCHUNK + PAD], 0.0)
            qkvT_tiles[c] = qkvT

            # xd^T = down @ x^T  [16, TCHUNK]
            ps_xd = ps_lin.tile([128, 512], F32, tag="lin",
                                name="ps_xd")[0:R, 0:TCHUNK]
            for ki in range(3):
                nc.tensor.matmul(ps_xd, downT[:, ki, :], xt[:, ki, :],
                                 start=(ki == 0), stop=(ki == 2))
            xd = sb_p.tile([R, TCHUNK], BF, tag="xd", name="xd")
            nc.scalar.copy(xd[:], ps_xd[:])

            for mi in range(9):
                ps_g = ps_lin.tile([128, 512], F32, tag="lin",
                                   name="ps_g")[:, 0:TCHUNK]
                if USE_DR:
                    for j in range(2):
                        nc.tensor.matmul(ps_g,
                                         wgf8[:, 2 * j:2 * j + 2,
                                              128 * mi:128 * mi + 128],
                                         xf8[:, 2 * j:2 * j + 2, :],
                                         start=(j == 0), stop=(j == 1),
                                         perf_mode=DR)
                else:
                    for ki in range(3):
                        nc.tensor.matmul(ps_g,
                                         wgT[:, ki, 128 * mi:128 * mi + 128],
                                         xt[:, ki, :],
                                         start=(ki == 0), stop=(ki == 2))
                # sigmoid(z) = 0.5*(1 + tanh(z/2)); tanh shares the exp
                # activation table (no ACT_TABLE_LOAD ping-pong). The 0.5
                # is folded into up_eff on the host.
                g = sb_p.tile([128, TCHUNK], BF, tag="g", name="g")
                nc.scalar.activation(g[:], ps_g[:], TANH,
                                     scale=(0.5 / GS if USE_DR else 0.5))

                ps_l = ps_lin.tile([128, 512], F32, tag="lin",
                                   name="ps_l")[:, 0:TCHUNK]
                nc.tensor.matmul(ps_l, upT[:, 128 * mi:128 * mi + 128], xd[:],
                                 start=True, stop=True)

                ps_m = ps_lin.tile([128, 512], F32, tag="lin",
                                   name="ps_m")[:, 0:TCHUNK]
                for ki in range(3):
                    nc.tensor.matmul(ps_m,
                                     wqkvT[:, ki, 128 * mi:128 * mi + 128],
                                     xt[:, ki, :],
                                     start=(ki == 0), stop=(ki == 2))
                gl = sb_p.tile([128, TCHUNK], BF, tag="gl", name="gl")
                # gl = (tanh + 1) * (0.5*lora)
                nc.vector.scalar_tensor_tensor(gl[:], g[:], 1.0, ps_l[:],
                                               ADD, MULT)
                # qkvT = (ps_m + bias_col) + g*lora
                nc.vector.scalar_tensor_tensor(
                    qkvT[:, mi, 0:TCHUNK], ps_m[:], bcols[:, mi:mi + 1],
                    gl[:], ADD, ADD)

        def emit_B_front(c, p):
            """attention pair p of chunk c: scores, exp, expb fold, V^T."""
            qkvT = qkvT_tiles[c]
            if p == 0:
                oT_tiles[c] = oT_p.tile([128, 3, TCHUNK], BF, name="oT")
            pc0 = p * NP2

            # One PSUM tile (bank) per score group so pair p+1's QK only
            # waits on exp of pair p for that bank. Score rows 98-127 are
            # zeros (FWL-padded stationaries) and ignored.
            ps = ps_s_p.tile([128, SB, 512], F32, tag="s", name="ps_s")
            pss = [ps[:, b] for b in range(SB)]
            if has_mask:
                bm = attn_p.tile([NP2, SB, SW], BF, tag="bm", name="bm")
                pm = (c * NPAIR + p) % 32
                nc.sync.dma_start(out=bm, in_=bmask_d[pm])
            else:
                bm = bmask_c
            # bias(+mask) logits -> PSUM via PE (ident stationary,
            # start=True sets has_written; cross-window blocks are -1e30
            # so exp kills them after QK accumulates on top).
            # ident cols 98-127 are zero in rows 0-97, so a 128-col
            # stationary (FWL) just writes zero rows 98-127.
            for b in range(SB):
                nc.tensor.matmul(pss[b][:, 0:SW], ident[0:NP2, 0:128],
                                 bm[:, b, :], start=True, stop=False,
                                 skip_group_check=True)

            # QK: one matmul per head, S^T[key, query], accumulating onto
            # the bias preload. With SB=4 the PSUM bank equals the PE row
            # group, so the 4 heads of a round run concurrently in
            # disjoint PE row groups + PSUM banks.
            if SB == 4:
                order = list(range(12))
            else:
                order = [4 * ((r + b) % 4) + b for r in range(4)
                         for b in range(3)]
            for h in order:
                s = h % 4
                nc.tensor.matmul(
                    pss[h_bank(h)][:, 98 * h_slot(h):98 * h_slot(h) + 98],
                    qkvT[32 * s:32 * s + 32, 3 + h // 4, pc0:pc0 + 128],
                    qkvT[32 * s:32 * s + 32, h // 4, pc0:pc0 + NP2],
                    start=False, stop=(h_slot(h) == 2),
                    tile_position=(32 * s, 0),
                    skip_group_check=True)

            # V transpose -> key-rows [98, 12, 32] (+ ones col for rowsum)
            v_ext = v_exts[p % 2]
            ps_vt = ps_vt_p.tile([128, 3, 128], BF, tag="vt", name="ps_vt")
            for ki in range(3):
                nc.tensor.transpose(ps_vt[:, ki, :],
                                    qkvT[:, 6 + ki, pc0:pc0 + 128],
                                    ident[:])
            nc.vector.tensor_copy(
                v_ext[:, :, 0:HD],
                ps_vt[0:NP2].rearrange("p a (b c) -> p (a b) c", c=HD))

            # exp per bank, reading PSUM directly (bias already folded).
            # 32 zero pad cols per bank so AV stationaries are 128 (FWL).
            ep = attn_p.tile([NP2, SB, SW + PAD], BF, tag="ep", name="ep")
            nc.vector.memset(ep[:, :, SW:SW + PAD], 0.0)
            for b in range(SB):
                nc.scalar.activation(ep[:, b, 0:SW], pss[b][0:NP2, 0:SW],
                                     EXP)
            ep_effs[p % 2] = ep

        def emit_B_av(c, p):
            """AV + normalize for pair p (emitted after front(p+1) so the
            exp -> gpsimd chain is hidden behind pair p+1's QK)."""
            ep_eff = ep_effs[p % 2]
            v_ext = v_exts[p % 2]
            # AV: one matmul per head, K=98; col 32 accumulates row sums.
            # Stationary ep_eff slice is 128 cols (FWL); output rows
            # 98-127 are garbage and ignored.
            ps_o_t = ps_o_p.tile([128, 512], F32, tag="o", name="ps_o")
            pov = ps_o_t[:, 0:H * (HD + 1)].rearrange("p (h c) -> p h c",
                                                      c=HD + 1)
            for h in range(H):
                nc.tensor.matmul(
                    pov[:, h, :],
                    ep_eff[:, h_bank(h), 98 * h_slot(h):98 * h_slot(h) + 128],
                    v_ext[:, h, :],
                    start=True, stop=True)

            r_t = attn_p.tile([NP2, H, 1], F32, tag="r", name="r")
            nc.vector.reciprocal(r_t[:], pov[0:NP2, :, HD:HD + 1])
            o_sb = o_sbs[p % 2]
            nc.vector.tensor_tensor(o_sb[:], pov[0:NP2, :, 0:HD],
                                    r_t.to_broadcast([NP2, H, HD]), MULT)

        def emit_B_back(c, p):
            """O^T transposes for pair p -> oT channel-rows."""
            oT = oT_tiles[c]
            pc0 = p * NP2
            o_sb = o_sbs[p % 2]
            ps_ot = ps_vt_p.tile([128, 3, 128], BF, tag="vt", name="ps_ot")
            for ki in range(3):
                nc.tensor.transpose(ps_ot[:, ki, 0:NP2],
                                    o_sb[:, 4 * ki:4 * ki + 4, :],
                                    ident[0:NP2, 0:NP2])
            nc.scalar.copy(oT[:, :, pc0:pc0 + NP2], ps_ot[:, :, 0:NP2])

        def emit_B(c):
            # software-pipeline: OT of pair p emitted after front of p+1
            emit_B_front(c, 0)
            emit_B_av(c, 0)
            emit_B_front(c, 1)
            emit_B_av(c, 1)
            emit_B_back(c, 0)
            emit_B_front(c, 2)
            emit_B_av(c, 2)
            emit_B_back(c, 1)
            emit_B_front(c, 3)
            emit_B_av(c, 3)
            emit_B_back(c, 2)
            emit_B_back(c, 3)

        def emit_C(c):
            """proj LoRA-linear for chunk c."""
            t0 = c * TCHUNK
            oT = oT_tiles.pop(c)
            qkvT_tiles.pop(c, None)
            ps_pxd = ps_lin.tile([128, 512], F32, tag="lin",
                                 name="ps_pxd")[0:R, 0:TCHUNK]
            for ki in range(3):
                nc.tensor.matmul(ps_pxd, pdownT[:, ki, :], oT[:, ki, :],
                                 start=(ki == 0), stop=(ki == 2))
            pxd = sb_p.tile([R, TCHUNK], BF, tag="xd", name="pxd")
            nc.scalar.copy(pxd[:], ps_pxd[:])

            out_sb = out_p.tile([128, 3, TCHUNK], F32, name="out_sb")
            for mi in range(3):
                ps_g2 = ps_lin.tile([128, 512], F32, tag="lin",
                                    name="ps_g2")[:, 0:TCHUNK]
                for ki in range(3):
                    nc.tensor.matmul(ps_g2,
                                     pgT[:, ki, 128 * mi:128 * mi + 128],
                                     oT[:, ki, :],
                                     start=(ki == 0), stop=(ki == 2))
                g2 = sb_p.tile([128, TCHUNK], BF, tag="g", name="g2")
                nc.scalar.activation(g2[:], ps_g2[:], TANH, scale=0.5)

                ps_l2 = ps_lin.tile([128, 512], F32, tag="lin",
                                    name="ps_l2")[:, 0:TCHUNK]
                nc.tensor.matmul(ps_l2, pupT[:, 128 * mi:128 * mi + 128],
                                 pxd[:], start=True, stop=True)

                ps_m2 = ps_lin.tile([128, 512], F32, tag="lin",
                                    name="ps_m2")[:, 0:TCHUNK]
                for ki in range(3):
                    nc.tensor.matmul(ps_m2,
                                     pwT[:, ki, 128 * mi:128 * mi + 128],
                                     oT[:, ki, :],
                                     start=(ki == 0), stop=(ki == 2))
                gl2 = sb_p.tile([128, TCHUNK], BF, tag="gl", name="gl2")
                nc.vector.scalar_tensor_tensor(gl2[:], g2[:], 1.0, ps_l2[:],
                                               ADD, MULT)
                nc.vector.scalar_tensor_tensor(
                    out_sb[:, mi, :], ps_m2[:], bcols[:, 9 + mi:10 + mi],
                    gl2[:], ADD, ADD)

            nc.sync.dma_start(out=outT_d[:, :, t0:t0 + TCHUNK], in_=out_sb)

        for c in range(NCHUNK):
            emit_A(c)
            if c > 0:
                emit_B(c - 1)
                emit_C(c - 1)
        emit_B(NCHUNK - 1)
        emit_C(NCHUNK - 1)

        for pool in reversed((consts, xt_p, xf8_p, qkvT_p, sb_p, attn_p,
                              oT_p, out_p, ps_lin, ps_s_p, ps_o_p, ps_vt_p)):
            pool.release()

    nc.compile()
    return nc


def _get_nc(has_mask: bool):
    key = (has_mask, USE_DR)
    if key not in _COMPILED:
        _COMPILED[key] = _build(has_mask)
    return _COMPILED[key]


def _arr_lhsT(w_t, kparts):
    """[K, M] -> [128, K//128, M] partition-tiled lhsT layout."""
    K, M = w_t.shape
    return np.ascontiguousarray(
        w_t.reshape(kparts, 128, M).transpose(1, 0, 2))


def _prep_inputs(x, mask, qkv_w, qkv_b, qkv_down, qkv_up, qkv_gate, qkv_res,
                 proj_w, proj_b, proj_down, proj_up, proj_gate, proj_res,
                 bias_table, rel_index):
    x = np.asarray(x, np.float32)
    mask = np.asarray(mask, np.float32)
    has_mask = bool(np.any(mask))

    w_eff = (np.asarray(qkv_w, np.float32)
             + np.asarray(qkv_res, np.float32))        # [1152, 384]
    # 0.5 factor: gate computed as 0.5*(1+tanh(z/2)) on device; the 0.5
    # is folded into the lora-up weights here.
    up_eff = np.asarray(qkv_up, np.float32) * (SCALING * 0.5)  # [1152, 16]
    b_eff = np.asarray(qkv_b, np.float32).copy()
    # fold attention scale into the q-channel outputs
    w_eff[0:D] *= SCALE
    up_eff[0:D] *= SCALE
    b_eff[0:D] *= SCALE

    pw_eff = (np.asarray(proj_w, np.float32)
              + np.asarray(proj_res, np.float32))
    pup_eff = np.asarray(proj_up, np.float32) * (SCALING * 0.5)

    # fp8 gate weights, 64x-scaled, K padded 384 -> 512 for DoubleRow
    gate_t = np.asarray(qkv_gate, np.float32).T * GS    # [384, 1152]
    gpad = np.zeros((512, 3 * D), np.float32)
    gpad[0:D] = gate_t
    wgf8 = np.ascontiguousarray(
        gpad.reshape(4, 128, 3 * D).transpose(1, 0, 2)).astype(FP8)

    bcols = np.zeros((128, 12), np.float32)
    bcols[:, 0:9] = b_eff.reshape(9, 128).T
    bcols[:, 9:12] = np.asarray(proj_b, np.float32).reshape(3, 128).T

    common = {
        "wqkvT": _arr_lhsT(w_eff.T, 3).astype(BF16),
        "wgf8": wgf8,
        "wgT": _arr_lhsT(np.asarray(qkv_gate, np.float32).T, 3).astype(BF16),
        "downT": _arr_lhsT(np.asarray(qkv_down, np.float32).T, 3).astype(BF16),
        "upT": np.ascontiguousarray(up_eff.T).astype(BF16),
        "pwT": _arr_lhsT(pw_eff.T, 3).astype(BF16),
        "pgT": _arr_lhsT(np.asarray(proj_gate, np.float32).T, 3).astype(BF16),
        "pdownT": _arr_lhsT(np.asarray(proj_down, np.float32).T, 3).astype(BF16),
        "pupT": np.ascontiguousarray(pup_eff.T).astype(BF16),
        "bcols": bcols,
        "ident": np.eye(128, dtype=BF16),
    }

    # score-logit preload in S^T layout: bmask[m, b, 98*s + n] for head
    # h = 4*b + s; -1e30 on cross-window blocks
    bt = np.asarray(bias_table, np.float32)
    ri = np.asarray(rel_index).astype(np.int64)
    b_nmh = bt[ri]                                # [n, m, H]

    def _bm(lg0, lg1):
        """lg_w [n, m, H] per window -> [98, SB, SW] preload tile."""
        big = np.full((NP2, NP2, H), -1e30, np.float32)   # [m, n, H]
        big[0:N, 0:N] = lg0.transpose(1, 0, 2)
        big[N:NP2, N:NP2] = lg1.transpose(1, 0, 2)
        mhn = big.transpose(0, 2, 1)                      # [m, H, n]
        if QK_BANKS == 3:
            # h = 4*bank + slot -> [m, bank, slot, n]
            arr = mhn.reshape(NP2, 3, 4, NP2)
        else:
            # h = 4*slot + bank -> [m, slot, bank, n] -> [m, bank, slot, n]
            arr = mhn.reshape(NP2, 3, 4, NP2).transpose(0, 2, 1, 3)
        sw = (12 // QK_BANKS) * NP2
        return np.ascontiguousarray(arr.reshape(NP2, QK_BANKS, sw))

    sw = (12 // QK_BANKS) * NP2
    if has_mask:
        bmask = np.zeros((32, NP2, QK_BANKS, sw), np.float32)
        for pm in range(32):
            lg0 = b_nmh + mask[2 * pm][:, :, None]
            lg1 = b_nmh + mask[2 * pm + 1][:, :, None]
            bmask[pm] = _bm(lg0, lg1)
    else:
        bmask = _bm(b_nmh, b_nmh)[None]
    common["bmask"] = bmask.astype(BF16)

    in_maps = []
    for core in range(NCORES):
        tok = np.ascontiguousarray(
            x[core * WPC:(core + 1) * WPC].reshape(TPC, D))
        xt = np.ascontiguousarray(
            tok.reshape(TPC, 3, 128).transpose(2, 1, 0)).astype(BF16)
        tpad = np.zeros((TPC, 512), np.float32)
        tpad[:, 0:D] = tok
        xf8 = np.ascontiguousarray(
            tpad.reshape(TPC, 4, 128).transpose(2, 1, 0)).astype(FP8)
        m = dict(common)
        m["xt"] = xt
        m["xf8"] = xf8
        in_maps.append(m)
    return has_mask, in_maps


def kernel(**inputs):
    has_mask, in_maps = _prep_inputs(**inputs)
    nc = _get_nc(has_mask)
    res = run_bass_kernel_spmd(nc, in_maps, list(range(NCORES)))
    outs = []
    for core in range(NCORES):
        ot = res.results[core]["outT"]            # [128, 3, TPC] f32
        out = np.ascontiguousarray(ot.transpose(2, 1, 0)).reshape(TPC, D)
        outs.append(out)
    full = np.concatenate(outs, axis=0).reshape(B_, N, D)
    return full.astype(np.float32)


def run_traced(**inputs):
    """Like kernel() but with NTFF profiling; returns (out, BassKernelResults)."""
    sys.path.insert(0, "/root/problem")
    import profhook
    profhook.install()
    has_mask, in_maps = _prep_inputs(**inputs)
    nc = _get_nc(has_mask)
    res = run_bass_kernel_spmd(nc, in_maps, list(range(NCORES)), trace=True)
    outs = []
    for core in range(NCORES):
        ot = res.results[core]["outT"]
        out = np.ascontiguousarray(ot.transpose(2, 1, 0)).reshape(TPC, D)
        outs.append(out)
    full = np.concatenate(outs, axis=0).reshape(B_, N, D)
    return full.astype(np.float32), res

